# revision 29
# baseline (speedup 1.0000x reference)
"""DiffLogicLayer Trainium2 kernel (host-gather sharding + fp16 streaming,
phase-serialized DMA).

Math: for each output neuron o with inputs a = x[:, ia[o]], b = x[:, ib[o]],
the 16 relaxed binary gates are all linear in {1, a, b, a*b}:

    gate_k(a, b) = C[k,0] + C[k,1]*a + C[k,2]*b + C[k,3]*a*b

so with w = softmax(weights[o]) the layer output collapses to

    out[n, o] = W0[o] + W1[o]*a + W2[o]*b + W3[o]*a*b,   W = softmax(weights) @ C

Sharding: tensor-parallel over out_dim (1024 neurons/core). The gather
x[:, idx] is pure data movement, so it is folded into the host-side input
sharding: each core receives its 2048 gathered rows of x^T pre-packed fp16.

Measured on this part: HBM reads alone sustain ~440 GB/s, writes ~360,
mixed ~330. So ALL loads and ALL stores go on the SAME sync-HWDGE ring:
FIFO drain => loads stream solo at read bandwidth, stores (queued behind,
gated on compute sems) drain after. ~19 DMAs avoids completion-semaphore
lane cross-gating (v3 lesson); the 8 lanes are GLOBAL across rings, and
the scalar HWDGE ring's first DMA completes ~5us late (cold start), so
everything stays on the sync ring (v14 lesson). GPSIMD is NOT used for
elementwise work: it shares SBUF ports with DVE and slows concurrent DVE
ops ~4x (v5 lesson). Ops stay full-tile (128, 2048): sliced/strided DVE
APs lose the 2x/4x perf modes (v7 lesson). scalar_tensor_tensor and the
custom DVE ops (AFFINE_*) run 1x-only (no 2x uops) — fusing v+o or u+t
through them is net slower (v13 lesson).

Compute per block: u = W3*a + W2 (ACT), v = W1*a + W0 (DVE tensor_scalar,
4x fp16), t = u*b, o = t + v (DVE tensor_tensor, 2x fp16). The DVE chain
is the critical path (~26us); it starts as soon as g0's completion sem
fires. Softmax+C-fold is fused via stride-0 broadcast APs over a 16KB C
constant so W0..W3 are ready before g0 lands. Block 7 splits a7 (loaded
right after g0; u7/v7 prepped in mid-stream slack) from b7 (loaded last;
only t+o+store remain at the tail).

Output fp16; host concatenates, transposes, casts to fp32. Max rel err vs
fp32 reference ~4e-3 (tolerance 2e-2).
"""

import os
import sys

import numpy as np

sys.path.insert(0, "/opt/trn_rl_repo")

import concourse.bacc as bacc
import concourse.mybir as mybir
from concourse import tile
from concourse.bass import broadcast_tensor_aps
from concourse.bass_utils import run_bass_kernel_spmd

AF = mybir.ActivationFunctionType
ALU = mybir.AluOpType
AX = mybir.AxisListType
F32 = mybir.dt.float32
F16 = mybir.dt.float16

IN_DIM = 8192
OUT_DIM = 8192
BATCH = 2048
N_CORES = 8
OPC = OUT_DIM // N_CORES  # 1024 neurons per core
NBLK = OPC // 128  # 8 partition blocks per core
HB = BATCH // 2

# gate_k = C[k,0] + C[k,1]*a + C[k,2]*b + C[k,3]*ab  (difflogic convention)
_C = np.array(
    [
        [0, 0, 0, 0],  # False
        [0, 0, 0, 1],  # a AND b
        [0, 1, 0, -1],  # a AND NOT b
        [0, 1, 0, 0],  # a
        [0, 0, 1, -1],  # NOT a AND b
        [0, 0, 1, 0],  # b
        [0, 1, 1, -2],  # XOR
        [0, 1, 1, -1],  # OR
        [1, -1, -1, 1],  # NOR
        [1, -1, -1, 2],  # XNOR
        [1, 0, -1, 0],  # NOT b
        [1, 0, -1, 1],  # a OR NOT b
        [1, -1, 0, 0],  # NOT a
        [1, -1, 0, 1],  # NOT a OR b
        [1, 0, 0, -1],  # NAND
        [1, 0, 0, 0],  # True
    ],
    dtype=np.float32,
)

_PROGRAM = None


def _build_program():
    nc = bacc.Bacc("TRN2", target_bir_lowering=False, debug=False)

    wpre = nc.dram_tensor("wpre", (128, NBLK * 16), F32, kind="ExternalInput")
    cb64 = nc.dram_tensor("cb64", (128, 4 * 16), F32, kind="ExternalInput")
    ga7 = nc.dram_tensor("ga7", (128, BATCH), F16, kind="ExternalInput")
    g0h = [
        nc.dram_tensor(f"g0h{h}", (128, BATCH), F16, kind="ExternalInput") for h in range(2)
    ]
    gblk = [
        nc.dram_tensor(f"g{j}", (128, 2 * BATCH), F16, kind="ExternalInput")
        for j in range(1, NBLK - 1)
    ]
    gb7 = nc.dram_tensor("gb7", (128, BATCH), F16, kind="ExternalInput")
    y0h = [
        nc.dram_tensor(f"y0h{h}", (128, HB), F16, kind="ExternalOutput") for h in range(2)
    ]
    ys = [
        nc.dram_tensor(f"y{j}", (128, BATCH), F16, kind="ExternalOutput")
        for j in range(1, NBLK)
    ]

    with tile.TileContext(nc) as tc:
        with (
            tc.tile_pool(name="const", bufs=1) as cpool,
            tc.tile_pool(name="gath", bufs=1) as gpool,
            tc.tile_pool(name="work", bufs=3) as wpool,
            tc.tile_pool(name="outp", bufs=1) as opool,
        ):
            # ---- loads: all on the sync HWDGE ring, in stream order ----
            wpre_t = cpool.tile([128, NBLK * 16], F32)
            nc.sync.dma_start(wpre_t[:, :], wpre[:, :])
            cb64_t = cpool.tile([128, 4 * 16], F32)
            nc.sync.dma_start(cb64_t[:, :], cb64[:, :])
            # block 0 as two 512KB column-half loads: the first half's
            # completion sem fires ~1.3us before a full-1MB g0 would, so the
            # DVE chain starts earlier.
            g0h_t = []
            for h in range(2):
                t = gpool.tile([128, BATCH], F16, name=f"g0h{h}", tag=f"g0h{h}")
                nc.sync.dma_start(t[:, :], g0h[h][:, :])
                g0h_t.append(t)
            # a7 loads after g2 (it is not needed until the u7/v7 prep,
            # which runs after block 3): keeps g1/g2 arriving ~1.2us earlier,
            # closing the measured DVE wait-for-g1 gap.
            g_t = []
            ga7_t = None
            for j in range(1, NBLK - 1):
                t = gpool.tile([128, 2 * BATCH], F16, tag=f"g{j}")
                nc.sync.dma_start(t[:, :], gblk[j - 1][:, :])
                g_t.append(t)
                if j == 2:
                    ga7_t = gpool.tile([128, BATCH], F16, tag="ga7")
                    nc.sync.dma_start(ga7_t[:, :], ga7[:, :])
            gb7_t = gpool.tile([128, BATCH], F16, tag="gb7")
            nc.sync.dma_start(gb7_t[:, :], gb7[:, :])

            # ---- softmax over the 16 gate logits + C-fold, fused ----
            e_t = cpool.tile([128, NBLK * 16], F32)
            nc.scalar.activation(e_t[:, :], wpre_t[:, :], AF.Exp)
            s_t = cpool.tile([128, NBLK], F32)
            nc.vector.tensor_reduce(
                s_t[:, :], e_t[:, :].rearrange("p (j k) -> p j k", k=16), AX.X, op=ALU.add
            )
            r_t = cpool.tile([128, NBLK], F32)
            nc.vector.reciprocal(r_t[:, :], s_t[:, :])
            # tmp[p, c, j, k] = e[p, j, k] * C[k, c]  (e bcast over c, C over j)
            tmp_t = cpool.tile([128, 4 * NBLK * 16], F32)
            tmp4 = tmp_t[:, :].rearrange("p (c j k) -> p c j k", c=4, k=16)
            e4 = e_t[:, :].rearrange("p (c j k) -> p c j k", c=1, k=16)
            e4b = broadcast_tensor_aps(tmp4, e4)[1]
            cbj = cb64_t[:, :].rearrange("p (c j k) -> p c j k", c=4, k=16)
            cbjb = broadcast_tensor_aps(tmp4, cbj)[1]
            nc.vector.tensor_tensor(tmp4, e4b, cbjb, op=ALU.mult)
            raw_t = cpool.tile([128, 4 * NBLK], F32)
            nc.vector.tensor_reduce(
                raw_t[:, :], tmp_t[:, :].rearrange("p (cj k) -> p cj k", k=16), AX.X, op=ALU.add
            )
            # w4 = raw * (1/s), with 1/s broadcast over c (stride-0)
            w4_t = cpool.tile([128, 4 * NBLK], F32)
            w43 = w4_t[:, :].rearrange("p (c j) -> p c j", c=4)
            r43 = r_t[:, :].rearrange("p (c j) -> p c j", c=1)
            r43b = broadcast_tensor_aps(w43, r43)[1]
            nc.vector.tensor_tensor(
                w43, raw_t[:, :].rearrange("p (c j) -> p c j", c=4), r43b, op=ALU.mult
            )

            def wc(c, j):
                return w4_t[:, c * NBLK + j : c * NBLK + j + 1]

            jl = NBLK - 1
            u7_t = gpool.tile([128, BATCH], F16, tag="u7")
            v7_t = gpool.tile([128, BATCH], F16, tag="v7")

            o0h_t = [
                opool.tile([128, HB], F16, name=f"o0h{h}", tag=f"o0h{h}") for h in range(2)
            ]
            o_t = [None] + [
                opool.tile([128, BATCH], F16, name=f"o{j}", tag=f"o{j}")
                for j in range(1, NBLK)
            ]

            # ---- block 0 (two column-halves, earliest data) ----
            for h in range(2):
                a_ap = g0h_t[h][:, 0:HB]
                b_ap = g0h_t[h][:, HB:BATCH]
                u_t = wpool.tile([128, HB], F16, name=f"u0h{h}", tag="u0h")
                v_t = wpool.tile([128, HB], F16, name=f"v0h{h}", tag="v0h")
                t_t = wpool.tile([128, HB], F16, name=f"t0h{h}", tag="t0h")
                nc.scalar.activation(u_t[:, :], a_ap, AF.Identity, bias=wc(2, 0), scale=wc(3, 0))
                nc.vector.tensor_scalar(
                    v_t[:, :], a_ap, wc(1, 0), wc(0, 0), op0=ALU.mult, op1=ALU.add
                )
                nc.vector.tensor_tensor(t_t[:, :], u_t[:, :], b_ap, op=ALU.mult)
                nc.vector.tensor_tensor(o0h_t[h][:, :], t_t[:, :], v_t[:, :], op=ALU.add)

            # ---- blocks 1..6: streaming compute ----
            for j in range(1, NBLK - 1):
                a_ap = g_t[j - 1][:, 0:BATCH]
                b_ap = g_t[j - 1][:, BATCH : 2 * BATCH]
                u_t = wpool.tile([128, BATCH], F16, tag="u")
                v_t = wpool.tile([128, BATCH], F16, tag="v")
                t_t = wpool.tile([128, BATCH], F16, tag="t")
                nc.scalar.activation(u_t[:, :], a_ap, AF.Identity, bias=wc(2, j), scale=wc(3, j))
                nc.vector.tensor_scalar(
                    v_t[:, :], a_ap, wc(1, j), wc(0, j), op0=ALU.mult, op1=ALU.add
                )
                nc.vector.tensor_tensor(t_t[:, :], u_t[:, :], b_ap, op=ALU.mult)
                nc.vector.tensor_tensor(o_t[j][:, :], t_t[:, :], v_t[:, :], op=ALU.add)
                if j == 3:
                    # block 7 affine prep in mid-stream slack (a7 landed by now)
                    nc.scalar.activation(
                        u7_t[:, :], ga7_t[:, :], AF.Identity, bias=wc(2, jl), scale=wc(3, jl)
                    )
                    nc.vector.tensor_scalar(
                        v7_t[:, :], ga7_t[:, :], wc(1, jl), wc(0, jl), op0=ALU.mult, op1=ALU.add
                    )

            # ---- block 7 tail: only t+o remain after b7 (last load) lands ----
            t7_t = wpool.tile([128, BATCH], F16, tag="t7")
            nc.vector.tensor_tensor(t7_t[:, :], u7_t[:, :], gb7_t[:, :], op=ALU.mult)
            nc.vector.tensor_tensor(o_t[jl][:, :], t7_t[:, :], v7_t[:, :], op=ALU.add)

            # ---- stores: SAME sync ring, queued behind all loads (FIFO) ----
            for h in range(2):
                nc.sync.dma_start(y0h[h][:, :], o0h_t[h][:, :])
            for j in range(1, NBLK):
                nc.sync.dma_start(ys[j - 1][:, :], o_t[j][:, :])

    nc.compile()
    return nc


def _get_program():
    global _PROGRAM
    if _PROGRAM is None:
        _PROGRAM = _build_program()
    return _PROGRAM


def make_in_maps(x, weights, indices_a, indices_b):
    x = np.asarray(x, dtype=np.float32)
    w = np.asarray(weights, dtype=np.float32)
    ia = np.asarray(indices_a).astype(np.int64)
    ib = np.asarray(indices_b).astype(np.int64)

    xt16 = np.ascontiguousarray(x.T.astype(np.float16))  # (IN_DIM, BATCH)

    cb64 = np.ascontiguousarray(
        np.broadcast_to(_C.T.reshape(1, 64), (128, 64)), dtype=np.float32
    )

    jl = NBLK - 1
    in_maps = []
    for c in range(N_CORES):
        sl = slice(c * OPC, (c + 1) * OPC)
        ia_c = ia[sl].reshape(NBLK, 128)
        ib_c = ib[sl].reshape(NBLK, 128)
        wsh = w[sl]  # (OPC, 16)
        m = {
            "cb64": cb64,
            "wpre": np.ascontiguousarray(
                wsh.reshape(NBLK, 128, 16).transpose(1, 0, 2).reshape(128, NBLK * 16)
            ),
        }
        a0, b0 = xt16[ia_c[0]], xt16[ib_c[0]]
        for h in range(2):
            half = np.empty((128, 2, HB), dtype=np.float16)
            half[:, 0, :] = a0[:, h * HB : (h + 1) * HB]
            half[:, 1, :] = b0[:, h * HB : (h + 1) * HB]
            m[f"g0h{h}"] = np.ascontiguousarray(half.reshape(128, BATCH))
        for j in range(1, NBLK - 1):
            blk = np.empty((128, 2, BATCH), dtype=np.float16)
            blk[:, 0, :] = xt16[ia_c[j]]
            blk[:, 1, :] = xt16[ib_c[j]]
            m[f"g{j}"] = np.ascontiguousarray(blk.reshape(128, 2 * BATCH))
        m["ga7"] = np.ascontiguousarray(xt16[ia_c[jl]])
        m["gb7"] = np.ascontiguousarray(xt16[ib_c[jl]])
        in_maps.append(m)
    return in_maps


def run(inputs, trace=False):
    if trace:
        try:
            from antenv.axon_hooks import get_axon_ntff_profile_hook  # noqa: F401
        except ImportError:
            trace = False
    nc = _get_program()
    in_maps = make_in_maps(
        inputs["x"], inputs["weights"], inputs["indices_a"], inputs["indices_b"]
    )
    res = run_bass_kernel_spmd(nc, in_maps, core_ids=list(range(N_CORES)), trace=trace)
    outT = np.empty((OUT_DIM, BATCH), dtype=np.float32)
    for c in range(N_CORES):
        r = res.results[c]
        base = c * OPC
        y0 = np.concatenate([r["y0h0"], r["y0h1"]], axis=1).astype(np.float32)
        outT[base : base + 128] = y0
        for j in range(1, NBLK):
            outT[base + j * 128 : base + (j + 1) * 128] = r[f"y{j}"].astype(np.float32)
    return np.ascontiguousarray(outT.T), res


def kernel(**inputs):
    out, _ = run(inputs, trace=bool(os.environ.get("DL_TRACE")))
    return out


if __name__ == "__main__":
    rng = np.random.default_rng(0)
    inputs = {
        "x": rng.random((BATCH, IN_DIM), dtype=np.float32),
        "weights": rng.standard_normal((OUT_DIM, 16)).astype(np.float32),
        "indices_a": rng.integers(0, IN_DIM, size=OUT_DIM),
        "indices_b": rng.integers(0, IN_DIM, size=OUT_DIM),
    }
    out = kernel(**inputs)
    print(out.shape, out.dtype)
